# revision 11
# baseline (speedup 1.0000x reference)
"""Binary TreeLSTM on 8 trn2 cores — v6.

vs v5: transposed W-stationary compute (gate dims on partitions, nodes
streaming), fp16 c-state/gates, 512B table rows (h bf16 | c fp16), HBM
transpose-gather that lands h/c already transposed for the matmul (no
consume-side PE transposes), per-partition bias folded into the ACT
instruction, results stored 16-bit and upcast on host. AllGather payload
shrinks 768B -> 512B per row.
"""

import numpy as np
import ml_dtypes

L, N, DIN, DOUT = 24, 8192, 256, 128
NCORES = 8
NS = N // NCORES
P = 128
BF16 = ml_dtypes.bfloat16
FP16 = np.float16

_CACHE = {}


def _build(levels, ns, n_cores):
    import concourse.bass as bass  # noqa: F401
    import concourse.bacc as bacc
    import concourse.tile as tile
    import concourse.mybir as mybir
    from concourse.masks import make_identity

    f32 = mybir.dt.float32
    bf16 = mybir.dt.bfloat16
    fp16 = mybir.dt.float16
    i16 = mybir.dt.int16
    SIG = mybir.ActivationFunctionType.Sigmoid
    TANH = mybir.ActivationFunctionType.Tanh

    T = ns // P                      # node tiles per core (8)
    NI = 2 * T * P                   # gathered rows per level (2048)
    V = n_cores * (ns + 1)           # table rows (8200)
    ROW = 256                        # u16 elems per row: h bf16 128 | c fp16 128
    G = 640                          # 5 gates x 128
    NG = 5                           # f1 f2 i o u
    CH = 512                         # node chunk for PSUM

    nc = bacc.Bacc("TRN2", target_bir_lowering=False, debug=False,
                   num_devices=n_cores, num_swdge_queues=2)

    xT_in = nc.dram_tensor("xT", [levels, DIN, ns], bf16, kind="ExternalInput")
    gidx_in = nc.dram_tensor("gidx16", [P, levels * (NI // 16)], i16,
                             kind="ExternalInput")
    Wp_in = nc.dram_tensor("Wp", [DIN, G], bf16, kind="ExternalInput")
    Ut_in = nc.dram_tensor("Ut", [2 * DOUT, G], bf16, kind="ExternalInput")
    bias_in = nc.dram_tensor("bias", [P, NG], f32, kind="ExternalInput")
    bias0_in = nc.dram_tensor("bias0", [P, NG], f32, kind="ExternalInput")
    cinit_in = nc.dram_tensor("cinitT", [P, 1], f32, kind="ExternalInput")
    initrow_in = nc.dram_tensor("initrow", [1, ROW], bf16, kind="ExternalInput")
    resh_out = nc.dram_tensor("resh", [levels, ns, DOUT], bf16, kind="ExternalOutput")
    resc_out = nc.dram_tensor("resc", [levels, ns, DOUT], fp16, kind="ExternalOutput")

    with tile.TileContext(nc) as tc:
        with (
            tc.tile_pool(name="const", bufs=1) as cp,
            tc.tile_pool(name="xp", bufs=3) as xp,
            tc.tile_pool(name="gp", bufs=2) as gp,
            tc.tile_pool(name="sp", bufs=2) as sp,
            tc.tile_pool(name="psum", bufs=6, space="PSUM") as psp,
            tc.tile_pool(name="ptr", bufs=2, space="PSUM") as ptp,
            tc.tile_pool(name="dram", bufs=2, space="DRAM") as dp,
        ):
            # --- constants ---
            Wp0 = cp.tile([P, G], bf16)
            Wp1 = cp.tile([P, G], bf16)
            Ut0 = cp.tile([P, G], bf16)
            Ut1 = cp.tile([P, G], bf16)
            nc.sync.dma_start(out=Wp0[:], in_=Wp_in[0:P, :])
            nc.sync.dma_start(out=Wp1[:], in_=Wp_in[P:2 * P, :])
            nc.sync.dma_start(out=Ut0[:], in_=Ut_in[0:P, :])
            nc.sync.dma_start(out=Ut1[:], in_=Ut_in[P:2 * P, :])
            bias_t = cp.tile([P, NG], f32)
            bias0_t = cp.tile([P, NG], f32)
            cinit_t = cp.tile([P, 1], f32)
            nc.sync.dma_start(out=bias_t[:], in_=bias_in[:])
            nc.sync.dma_start(out=bias0_t[:], in_=bias0_in[:])
            nc.sync.dma_start(out=cinit_t[:], in_=cinit_in[:])
            gidx_t = cp.tile([P, levels * (NI // 16)], i16)
            nc.sync.dma_start(out=gidx_t[:], in_=gidx_in[:])
            identb = cp.tile([P, P], bf16)
            identh = cp.tile([P, P], fp16)
            make_identity(nc, identb[:])
            make_identity(nc, identh[:])
            cinit_h = cp.tile([P, 1], fp16)
            nc.vector.tensor_copy(out=cinit_h[:], in_=cinit_t[:])

            cc_in = dp.tile([ns + 1, ROW], bf16, bufs=1)
            nc.sync.dma_start(out=cc_in[0:1, :], in_=initrow_in[:])

            prev_tbl = None
            for l in range(levels):
                xk0 = xp.tile([P, ns], bf16)
                xk1 = xp.tile([P, ns], bf16)
                nc.sync.dma_start(out=xk0[:], in_=xT_in[l, 0:P, :])
                nc.sync.dma_start(out=xk1[:], in_=xT_in[l, P:2 * P, :])

                if l > 0:
                    # row-gather children (512B rows) in 4 chunks split over
                    # 2 half-buffers so PE transposes pipeline with the tail
                    # of the gather
                    lcol = l * (NI // 16)
                    CHG = 512
                    ncol = CHG // 16
                    ghalf = [gp.tile([P, T * ROW], bf16, name=f"gh{h}")
                             for h in range(2)]
                    for k in range(NI // CHG):
                        g3 = ghalf[k // 2].rearrange("p (s r) -> p s r", r=ROW)
                        idxs = gidx_t[:, lcol + k * ncol: lcol + (k + 1) * ncol]
                        nc.gpsimd.dma_gather(
                            out_ap=g3[:, (k % 2) * (CHG // P):
                                      (k % 2 + 1) * (CHG // P), :],
                            in_ap=prev_tbl[:],
                            idxs_ap=idxs, num_idxs=CHG, num_idxs_reg=CHG,
                            elem_size=ROW, elem_step=ROW,
                            transpose=False, queue_num=k % 2,
                        )
                    hct = gp.tile([P, 4, ns], bf16)

                    def consume_transposes():
                        for t in range(T):
                            g4 = ghalf[t // 4].rearrange(
                                "p (t c r) -> p t c r", c=2, r=ROW)
                            tt = t % 4
                            ts = slice(t * P, (t + 1) * P)
                            trx = ptp.tile([P, 2 * ROW], bf16, space="PSUM")
                            nc.tensor.transpose(out=trx[:, 0:P],
                                                in_=g4[:, tt, 0, 0:P],
                                                identity=identb[:])
                            nc.tensor.transpose(out=trx[:, P:2 * P],
                                                in_=g4[:, tt, 1, 0:P],
                                                identity=identb[:])
                            nc.tensor.transpose(
                                out=trx[:, 2 * P:3 * P].bitcast(fp16),
                                in_=g4[:, tt, 0, P:ROW].bitcast(fp16),
                                identity=identh[:])
                            nc.tensor.transpose(
                                out=trx[:, 3 * P:4 * P].bitcast(fp16),
                                in_=g4[:, tt, 1, P:ROW].bitcast(fp16),
                                identity=identh[:])
                            nc.vector.tensor_copy(
                                out=hct[:, :, ts],
                                in_=trx.rearrange("p (a b) -> p a b", a=4))

                bb = bias0_t if l == 0 else bias_t
                gs = [sp.tile([P, ns], fp16, name=f"g{m}") for m in range(NG)]
                NCH = ns // CH

                # PSUM tiles per (chunk, gate); issue x-matmuls as early as
                # possible so they overlap the previous level's exchange.
                pg = [[None] * NG for _ in range(NCH)]

                def xmm(ch, m):
                    t = psp.tile([P, CH], f32, space="PSUM")
                    pg[ch][m] = t
                    cs = slice(ch * CH, (ch + 1) * CH)
                    nc.tensor.matmul(out=t[:], lhsT=Wp0[:, m * P:(m + 1) * P],
                                     rhs=xk0[:, cs], start=True, stop=False)
                    nc.tensor.matmul(out=t[:], lhsT=Wp1[:, m * P:(m + 1) * P],
                                     rhs=xk1[:, cs], start=False, stop=(l == 0))

                def hmm(ch, m):
                    t = pg[ch][m]
                    cs = slice(ch * CH, (ch + 1) * CH)
                    nc.tensor.matmul(out=t[:], lhsT=Ut0[:, m * P:(m + 1) * P],
                                     rhs=hct[:, 0, cs], start=False, stop=False)
                    nc.tensor.matmul(out=t[:], lhsT=Ut1[:, m * P:(m + 1) * P],
                                     rhs=hct[:, 1, cs], start=False, stop=True)

                def gate(ch, m):
                    func = TANH if m == 4 else SIG
                    nc.scalar.activation(
                        out=gs[m][:, ch * CH:(ch + 1) * CH], in_=pg[ch][m][:],
                        func=func, bias=bb[:, m:m + 1])

                if l == 0:
                    for ch in range(NCH):
                        for m in range(NG):
                            xmm(ch, m)
                            gate(ch, m)
                else:
                    # x-matmuls for 6 of 10 (ch,m) pairs run ahead of the
                    # gather; the rest follow as PSUM banks free up.
                    for m in range(NG):
                        xmm(0, m)
                    xmm(1, 0)
                    consume_transposes()
                    for m in range(NG):
                        hmm(0, m)
                        gate(0, m)
                        if m + 1 < NG:
                            xmm(1, m + 1)
                    for m in range(NG):
                        hmm(1, m)
                        gate(1, m)

                # --- elementwise (fp16, dims on partitions) ---
                if l > 0:
                    c0v = hct[:, 2, :].bitcast(fp16)
                    c1v = hct[:, 3, :].bitcast(fp16)
                else:
                    c0v = cinit_h[:].to_broadcast([P, ns])
                    c1v = c0v
                tiu = sp.tile([P, ns], fp16)
                t2 = sp.tile([P, ns], fp16)
                t3 = sp.tile([P, ns], fp16)
                c_new = sp.tile([P, ns], fp16)
                tnh = sp.tile([P, ns], fp16)
                h_new = sp.tile([P, ns], bf16)
                nc.vector.tensor_mul(out=tiu[:], in0=gs[2][:], in1=gs[4][:])
                nc.vector.tensor_mul(out=t2[:], in0=gs[0][:], in1=c0v)
                nc.vector.tensor_mul(out=t3[:], in0=gs[1][:], in1=c1v)
                nc.vector.tensor_add(out=t2[:], in0=t2[:], in1=t3[:])
                nc.vector.tensor_add(out=c_new[:], in0=tiu[:], in1=t2[:])
                nc.scalar.activation(out=tnh[:], in_=c_new[:], func=TANH)
                nc.vector.tensor_mul(out=h_new[:], in0=gs[3][:], in1=tnh[:])

                # --- transpose to node-major slab [p, t, h|c] ---
                slab = sp.tile([P, T, ROW], bf16)
                slab_h = slab.rearrange("p t r -> p (t r)")
                for t in range(T):
                    ts = slice(t * P, (t + 1) * P)
                    trx = ptp.tile([P, ROW], bf16, space="PSUM")
                    nc.tensor.transpose(out=trx[:, 0:P], in_=h_new[:, ts],
                                        identity=identb[:])
                    nc.tensor.transpose(out=trx[:, P:ROW].bitcast(fp16),
                                        in_=c_new[:, ts], identity=identh[:])
                    nc.vector.tensor_copy(out=slab[:, t, :], in_=trx[:])

                # --- outputs + exchange ---
                nc.sync.dma_start(
                    out=resh_out[l].rearrange("(t p) d -> p t d", p=P),
                    in_=slab[:, :, 0:P])
                nc.sync.dma_start(
                    out=resc_out[l].rearrange("(t p) d -> p t d", p=P),
                    in_=slab[:, :, P:ROW].bitcast(fp16))
                if l < levels - 1:
                    nc.sync.dma_start(
                        out=cc_in[1:1 + ns, :].rearrange("(t p) r -> p t r", p=P),
                        in_=slab[:])
                    tbl = dp.tile([V, ROW], bf16,
                                  addr_space="Shared" if n_cores > 4 else "Local")
                    nc.gpsimd.collective_compute(
                        "AllGather", mybir.AluOpType.bypass,
                        replica_groups=[list(range(n_cores))],
                        ins=[cc_in[:].opt()], outs=[tbl[:].opt()],
                    )
                    prev_tbl = tbl

    nc.compile()
    return nc


def _prep_shared(inputs):
    W_w, W_b = inputs["W_w"], inputs["W_b"]
    U_f1, U_f2, U_iuo = inputs["U_f1"], inputs["U_f2"], inputs["U_iuo"]
    h_init, c_init = inputs["h_init"], inputs["c_init"]
    D = DOUT
    Wt = np.asarray(W_w).T
    Wf, Wi, Wu, Wo = Wt[:, 0:D], Wt[:, D:2 * D], Wt[:, 2 * D:3 * D], Wt[:, 3 * D:4 * D]
    Wp = np.concatenate([Wf, Wf, Wi, Wo, Wu], axis=1)
    b = np.asarray(W_b)
    bp = np.concatenate([b[0:D], b[0:D], b[D:2 * D], b[3 * D:4 * D],
                         b[2 * D:3 * D]])
    Ut = np.concatenate([np.asarray(U_f1).T, np.asarray(U_f2).T,
                         np.asarray(U_iuo).T[:, 0:D],
                         np.asarray(U_iuo).T[:, 2 * D:3 * D],
                         np.asarray(U_iuo).T[:, D:2 * D]], axis=1)
    hc0 = np.concatenate([np.asarray(h_init), np.asarray(h_init)], axis=1)
    bp0 = bp + (hc0.astype(np.float64) @ Ut.astype(np.float64))[0]
    # [128, 5]: bias for gate g, dim d at [d, g]
    bias_t = np.ascontiguousarray(bp.reshape(5, D).T, np.float32)
    bias0_t = np.ascontiguousarray(bp0.reshape(5, D).T.astype(np.float32))
    hb = np.asarray(h_init).astype(BF16).reshape(-1).view(np.uint16)
    cb = np.asarray(c_init).astype(FP16).reshape(-1).view(np.uint16)
    initrow = np.concatenate([hb, cb])[None, :].view(BF16)
    cinitT = np.ascontiguousarray(np.asarray(c_init).reshape(-1, 1), np.float32)
    return dict(
        Wp=Wp.astype(BF16), Ut=Ut.astype(BF16),
        bias=bias_t, bias0=bias0_t, cinitT=cinitT,
        initrow=np.ascontiguousarray(initrow),
    )


def _prep_core(inputs, r, levels, ns):
    NI = 2 * ns
    x = np.asarray(inputs["tensor"])[:, r * ns:(r + 1) * ns, :]
    xT = np.ascontiguousarray(x.transpose(0, 2, 1)).astype(BF16)
    idx = np.asarray(inputs["indices"])[:, r * ns:(r + 1) * ns, :].astype(np.int64)
    rem = np.where(idx < 0, 0, (idx // ns) * (ns + 1) + 1 + (idx % ns))
    T = ns // P
    arr = rem.reshape(levels, T, P, 2).transpose(0, 1, 3, 2).reshape(levels, NI)
    blk = arr.reshape(levels, NI // 16, 16).transpose(0, 2, 1)
    g16 = np.tile(blk, (1, P // 16, 1)).transpose(1, 0, 2).reshape(P, levels * (NI // 16))
    return dict(xT=xT, gidx16=np.ascontiguousarray(g16, np.int16))


def _run(inputs, trace=False, levels=L, n_total=N, n_cores=NCORES):
    from concourse import bass_utils

    ns = n_total // n_cores
    key = (levels, ns, n_cores)
    if key not in _CACHE:
        _CACHE[key] = _build(levels, ns, n_cores)
    nc = _CACHE[key]

    shared = _prep_shared(inputs)
    in_maps = []
    for r in range(n_cores):
        m = dict(shared)
        m.update(_prep_core(inputs, r, levels, ns))
        in_maps.append(m)

    res = bass_utils.run_bass_kernel_spmd(
        nc, in_maps, core_ids=list(range(n_cores)), trace=trace)
    res_h = np.concatenate(
        [np.asarray(res.results[r]["resh"], np.float32) for r in range(n_cores)],
        axis=1)
    res_c = np.concatenate(
        [np.asarray(res.results[r]["resc"], np.float32) for r in range(n_cores)],
        axis=1)
    return res_h, res_c, res


def kernel(**inputs):
    res_h, res_c, _ = _run(inputs)
    return res_h, res_c
